# revision 40
# baseline (speedup 1.0000x reference)
"""Linformer-style multi-head attention on 8 Trainium2 NeuronCores.

Problem (hardcoded): B=4, S=4096, C=1024, H=16, D=64, DK=256, fp32 in/out.

Sharding: core i handles (batch b = i//2, head-group g = i%2 of 8 heads).
Each core computes its 8 heads' attention and the partial output
projection out_part = head_out_g @ Wo[:, g_cols].T (emitted bf16); the
host sums the two head-group partials per batch and adds bo.

Key algebraic reordering: K and V are NEVER materialized. The Linformer
projections commute with the head projections:
    Kp = Wk @ (x^T E^T),  Vp^T = (x^T F^T)^T @ Wv^T
so we first compute xEF = x^T @ [E^T | F^T]  ([C, 2*DK], contraction
over the full sequence), then tiny [C]-contractions produce Kp [hd, dk]
and Vp^T [dk, hd]. This replaces the x@Wk / x@Wv GEMMs + sequence
reductions (328k PE cycles) with 131k + 16k.

Phase schedule (single stream).  The golden rule on TRN2 is that ANY
tensor-engine idle gap >~1us drops the HAM p-state to K=4/8 (half
throughput) for the next 10-27us, so every phase boundary is bridged
with independent PE work:
  warmup  16 throwaway matmuls on a memset tile so the p-state ramps
          while the first input DMAs land.
  A||B    A = xEF accumulation in two c-half passes (4 PSUM banks each,
          pass 2 reuses pass 1's banks after an interleaved drain);
          B = Q^T = Wq x^T per (chunk, head-pair).  Units interleave so
          HBM demand stays ~200-260 GB/s instead of phase-serial
          380/70; const loads are spread across units in first-use
          order.  ef stays SBUF-resident for pass 2; x streams once.
  C       Kp/Vp from xEF, between two B-units; bias-adds (DVE/ACT) are
          off the critical path.
  ramp    the software pipeline's DEPTH stage_a's are ACT-bound (two
          1.1us exps each), so each is paired with a leftover B-unit as
          PE filler; their scores use scratch PSUM in the qt pool.
  D       attention + output projection, software-pipelined over
          (chunk, head-pair) items: scoresT via row-group-packed K=64
          matmul pairs, exp on ACT, AV+softmax-den via col-tiled M=64
          matmul pairs, one full-width reciprocal+mul on DVE, outproj
          spread one s-tile per item one chunk late.  The last five
          B-units ride the first five items (outps-pool scratch) as
          filler while the normalize chain fills.  osb copies split
          DVE/ACT; out partials written bf16.
  PSUM: warm 1 + qt 3 | A 4 + qt 3 | kvps 4 + qt 3 | ramp qt 3 + scpw 4
  | D: avps 2 + outps 2 + scps 4.
"""

import threading

import numpy as np

B, S, C = 4, 4096, 1024
H, D, DK = 16, 64, 256
HG = 8               # heads per core
HD = HG * D          # 512
NCORES = 8
EF = 2 * DK          # stacked E^T|F^T columns: 512
SCH = 512            # sequence chunk (phase B / D)
NCH = S // SCH       # 8 chunks
NST = SCH // 128     # 4 s-tiles per chunk
NCT = C // 128       # 8 c-tiles
NPT = HD // 128      # 4 hd blocks (head pairs)
NDB = DK // 128      # 2 dk blocks
NSG = 8              # phase-A s-groups (512 rows each, 4-row interleave)
CH2 = C // 2         # phase-A column half
DEPTH = 4            # phase-D software-pipeline lookahead
RAMP = 6             # stage_a items issued during the rampup

_lock = threading.Lock()
_compiled = None


def _build():
    import concourse.bacc as bacc
    import concourse.bass as bass
    import concourse.tile as tile
    from concourse import mybir

    F32 = mybir.dt.float32
    BF16 = mybir.dt.bfloat16
    EXP = mybir.ActivationFunctionType.Exp

    nc = bacc.Bacc(None, target_bir_lowering=False)

    xs = nc.dram_tensor("xs", [S, C], BF16, kind="ExternalInput")
    xT = nc.dram_tensor("xt", [C, S], BF16, kind="ExternalInput")
    efT = nc.dram_tensor("eft", [S, EF], BF16, kind="ExternalInput")
    wqT = nc.dram_tensor("wqt", [C, HD], BF16, kind="ExternalInput")
    wkT = nc.dram_tensor("wkt", [C, HD], BF16, kind="ExternalInput")
    wvT = nc.dram_tensor("wvt", [C, HD], BF16, kind="ExternalInput")
    eb = nc.dram_tensor("eb", [DK], F32, kind="ExternalInput")
    fb = nc.dram_tensor("fb", [DK], F32, kind="ExternalInput")
    woT = nc.dram_tensor("wot", [HD, C], BF16, kind="ExternalInput")
    ones = nc.dram_tensor("ones", [128, 64], BF16, kind="ExternalInput")
    out = nc.dram_tensor("out", [S, C], BF16, kind="ExternalOutput")

    # phase-A x/ef tiles: partition p carries 4 consecutive s-rows (fat
    # contiguous DRAM reads per partition).  The s-permutation is
    # irrelevant: phase A only ever CONTRACTS over s, with x and ef
    # sharing the same permutation.
    xs_r = xs[:].rearrange("(g p four) c -> g p four c", p=128, four=4)
    ef_r = efT[:].rearrange("(g p four) k -> g p four k", p=128, four=4)
    xT_r = xT[:].rearrange("(ct p) s -> ct p s", p=128)     # [8,128,4096]
    wq_r = wqT[:].rearrange("(ct p) n -> ct p n", p=128)    # [8,128,512]
    wk_r = wkT[:].rearrange("(ct p) n -> ct p n", p=128)
    wv_r = wvT[:].rearrange("(ct p) n -> ct p n", p=128)
    wo_r = woT[:].rearrange("(pt p) c -> pt p c", p=128)    # [4,128,1024]

    with tile.TileContext(nc) as tc:
        with (
            tc.tile_pool(name="consts", bufs=1) as consts,
            tc.tile_pool(name="mids", bufs=1) as mids,
            tc.tile_pool(name="pbx", bufs=3) as pbx,
            tc.tile_pool(name="p2ex", bufs=6) as p2ex,
            tc.tile_pool(name="p2ho", bufs=3) as p2ho,
            tc.tile_pool(name="p2rc", bufs=1) as p2rc,
            tc.tile_pool(name="p2out", bufs=4) as p2out,
        ):
            warm_sb = consts.tile([128, 512], BF16)
            wq_sb = consts.tile([128, NCT, HD], BF16)
            wk_sb = consts.tile([128, NCT, HD], BF16)
            wv_sb = consts.tile([128, NCT, HD], BF16)
            eb_sb = consts.tile([128, DK], F32)
            fb_sb = consts.tile([128, NDB], F32)
            ones_sb = consts.tile([128, 64], BF16)

            ef_sb = mids.tile([128, NSG, 4, EF], BF16)  # resident E^T|F^T
            xef_sb = mids.tile([128, NCT, EF], BF16)    # [c, 2dk]
            kp_sb = mids.tile([128, NPT, DK], BF16)     # Kp [hd, dk]
            vp_sb = mids.tile([128, NDB, HD], BF16)     # Vp^T [dk, hd]
            qt_sb = mids.tile([128, NCH * NPT, SCH], BF16)  # Q^T, all chunks
            wo_sb = mids.tile([128, NPT, C], BF16)

            items = [(ch, pt) for ch in range(NCH) for pt in range(NPT)]
            ex_tiles = {}
            ho_tiles = {}

            def stage_a(ch, pt, pool, tname):
                # one 4-bank scp tile and ONE exp over all 2048 columns:
                # the second activation's ~450ns fixed cost was pushing
                # ACT to the pipeline pace.
                qt_c = qt_sb[:, ch * NPT + pt, :]
                ex = p2ex.tile([128, 2, NDB, SCH], BF16, name="ex")
                scp = pool.tile([128, 2, NDB, SCH], F32, name=tname, bufs=1)
                for hrow in range(2):
                    lo, hi = hrow * 64, (hrow + 1) * 64
                    for j in range(NDB):
                        nc.tensor.matmul(
                            scp[:, hrow, j, :],
                            kp_sb[lo:hi, pt, j * 128:(j + 1) * 128],
                            qt_c[lo:hi, :],
                            start=True, stop=True,
                        )
                nc.scalar.activation(ex[:], scp, EXP, scale=0.125)
                return ex

            # qtps footprint = 3 x 1 bank, coexisting with warmps(1) /
            # accA(4) / kvps(4) / rampps(4) under the 8-bank budget.
            with tc.tile_pool(name="qtps", bufs=3, space="PSUM") as qtps:
                # ---- warmup: PE busy while the first input DMAs land --
                with tc.tile_pool(name="warmps", bufs=1, space="PSUM") as wps:
                    warm_ps = wps.tile([128, 512], F32)
                    nc.vector.memset(warm_sb[:], 0.0)
                    # ~7us of PE busy: cold HBM can't feed the first A
                    # units before then, and idle-starved starts drop
                    # the HAM p-state for 10-20us.
                    for _ in range(32):
                        nc.tensor.matmul(
                            warm_ps, warm_sb[:, 0:128], warm_sb,
                            start=True, stop=True,
                        )

                # ---- DMA kickoff (only what phase start needs) -------
                nc.scalar.dma_start(ones_sb, ones[:])
                for f in range(4):
                    nc.sync.dma_start(ef_sb[:, 0, f, :], ef_r[0][:, f, :])
                a_tiles = {}

                def fetch_a(p, g):
                    if (p, g) in a_tiles or g >= NSG:
                        return a_tiles.get((p, g))
                    t = pax.tile([128, 4, CH2], BF16, name="xsg")
                    if p == 0 and g == 0:
                        for f in range(4):
                            nc.sync.dma_start(
                                t[:, f, :], xs_r[0][:, f, 0:CH2]
                            )
                    else:
                        nc.sync.dma_start(
                            t[:], xs_r[g][:, :, p * CH2:(p + 1) * CH2]
                        )
                    a_tiles[(p, g)] = t
                    return t

                # gpsimd: xT per chunk.  SW-DGE issue (~1us/desc)
                # staggers the chunks; pbx bufs=3 WAR-paces the rest
                # (gpsimd has nothing else to do).
                xt_tiles = []
                for ch in range(NCH):
                    xt_t = pbx.tile([128, NCT, SCH], BF16, name="xtc")
                    for ct in range(NCT):
                        nc.gpsimd.dma_start(
                            xt_t[:, ct, :],
                            xT_r[ct, :, ch * SCH:(ch + 1) * SCH],
                        )
                    xt_tiles.append(xt_t)

                # consts spread across units on scalar, first-use order:
                # (pass, group) -> list of dma closures
                feed = {}

                def _later(p, g, fn):
                    feed.setdefault((p, g), []).append(fn)

                for ct in range(NCT):
                    _later(0, ct // 4, lambda ct=ct: nc.scalar.dma_start(
                        wq_sb[:, ct, :], wq_r[ct]))
                for ct in range(NCT):
                    _later(0, 4 + ct // 2, lambda ct=ct: nc.scalar.dma_start(
                        wk_sb[:, ct, :], wk_r[ct]))
                for ct in range(NCT):
                    _later(1, ct // 2, lambda ct=ct: nc.scalar.dma_start(
                        wv_sb[:, ct, :], wv_r[ct]))
                eb_bc = bass.AP(tensor=eb[:].tensor, offset=0, ap=[[0, 128], [1, DK]])
                _later(1, 1, lambda: nc.scalar.dma_start(eb_sb[:], eb_bc))

                def _fb(db):
                    fb_col = fb[db * 128:(db + 1) * 128].rearrange(
                        "(p one) -> p one", one=1
                    )
                    nc.scalar.dma_start(fb_sb[:, db:db + 1], fb_col)

                _later(1, 2, lambda: _fb(0))
                _later(1, 2, lambda: _fb(1))
                for pt in range(NPT):
                    _later(1, 4 + pt, lambda pt=pt: nc.scalar.dma_start(
                        wo_sb[:, pt, :], wo_r[pt]))

                def a_unit(p, g, xef_ps):
                    x_t = fetch_a(p, g)
                    if p == 0 and 2 <= g + 2 < NSG:
                        nc.sync.dma_start(ef_sb[:, g + 2, :, :], ef_r[g + 2])
                    fetch_a(p, g + 2)
                    if p == 0 and g >= NSG - 2:
                        fetch_a(1, g - (NSG - 2))
                    for fn in feed.get((p, g), []):
                        fn()
                    last_g = g == NSG - 1
                    for f in range(4):
                        first = g == 0 and f == 0
                        last = last_g and f == 3
                        for ct in range(4):
                            nc.tensor.matmul(
                                xef_ps[:, ct, :],
                                x_t[:, f, ct * 128:(ct + 1) * 128],
                                ef_sb[:, g, f, :],
                                start=first, stop=last,
                            )
                            if last:
                                # interleaved drain: bank ct final here;
                                # copy while the PE continues ct+1..
                                dst = xef_sb[:, p * 4 + ct, :]
                                if ct % 2 == 0:
                                    nc.vector.tensor_copy(dst, xef_ps[:, ct, :])
                                else:
                                    nc.scalar.copy(dst, xef_ps[:, ct, :])
                    a_tiles.pop((p, g))

                def b_unit(idx, pool, tname):
                    ch, pt = idx // NPT, idx % NPT
                    xt_t = xt_tiles[ch]
                    qps = pool.tile([128, SCH], F32, name=tname)
                    for ct in range(NCT):
                        nc.tensor.matmul(
                            qps,
                            wq_sb[:, ct, pt * 128:(pt + 1) * 128],
                            xt_t[:, ct, :],
                            start=(ct == 0), stop=(ct == NCT - 1),
                        )
                    nc.vector.tensor_copy(qt_sb[:, ch * NPT + pt, :], qps)

                # B-unit budget: 21 in A||B, 2 around C, 5 in the D
                # rampup, 4 spread into D's first items.  Pass-1 units
                # carry 3 B each (1A:3B keeps pass-1 HBM demand at
                # ~240 GB/s — pass 1 moves 2/3 of phase A's bytes);
                # pass 2 runs nearly A-only at ~150 GB/s.
                npair = {0: {1: 1, 2: 3, 3: 3, 4: 3, 5: 3, 6: 3, 7: 3},
                         1: {0: 1, 1: 1}}
                bi = 0
                with (
                    tc.tile_pool(name="pax", bufs=3) as pax,
                    tc.tile_pool(name="accA", bufs=1, space="PSUM") as accA,
                ):
                    fetch_a(0, 0)
                    fetch_a(0, 1)
                    nc.sync.dma_start(ef_sb[:, 1, :, :], ef_r[1])
                    for p in range(2):
                        xef_ps = accA.tile([128, 4, EF], F32, name="xefps")
                        for g in range(NSG):
                            a_unit(p, g, xef_ps)
                            for _ in range(npair[p].get(g, 0)):
                                b_unit(bi, qtps, "qps")
                                bi += 1

                # ---- C: Kp/Vp, bracketed by B-units ------------------
                with tc.tile_pool(name="kvps", bufs=1, space="PSUM") as kvps:
                    kp_ps = kvps.tile([128, NPT, DK], F32)      # 2 banks
                    vp_ps = kvps.tile([128, NDB, HD], F32)      # 2 banks
                    b_unit(bi, qtps, "qps")
                    bi += 1
                    for pt in range(NPT):
                        for ct in range(NCT):
                            nc.tensor.matmul(
                                kp_ps[:, pt, :],
                                wk_sb[:, ct, pt * 128:(pt + 1) * 128],
                                xef_sb[:, ct, 0:DK],
                                start=(ct == 0 and pt % 2 == 0),
                                stop=(ct == NCT - 1 and pt % 2 == 1),
                            )
                    for pt in range(NPT):
                        nc.vector.tensor_add(
                            kp_sb[:, pt, :], kp_ps[:, pt, :], eb_sb
                        )
                    b_unit(bi, qtps, "qps")
                    bi += 1
                    for db in range(NDB):
                        for ct in range(NCT):
                            nc.tensor.matmul(
                                vp_ps[:, db, :],
                                xef_sb[:, ct, DK + db * 128:DK + (db + 1) * 128],
                                wv_sb[:, ct, :],
                                start=(ct == 0), stop=(ct == NCT - 1),
                            )
                    for db in range(NDB):
                        # fb varies along the partition (dk) axis: ACT's
                        # per-partition bias-add fits, keeping the DVE
                        # free for the qt copies.
                        nc.scalar.add(
                            vp_sb[:, db, :], vp_ps[:, db, :],
                            fb_sb[:, db:db + 1],
                        )

                # ---- D rampup: stage_a is ACT-bound (~2us exp), so
                # pair each with a B-unit as PE filler; the B-unit
                # covers the previous item's exp (scp bufs=1).  Scores
                # use a dedicated 4-bank pool that closes before D's
                # scps opens; RAMP=5 so the main loop's first scps use
                # is one full item after the last ramp exp.
                with tc.tile_pool(name="rampps", bufs=1, space="PSUM") as rps:
                    for r in range(RAMP):
                        b_unit(bi, qtps, "qps")
                        bi += 1
                        ex_tiles[items[r]] = stage_a(*items[r], rps, "scpw")

            # ---------------- phase D main loop ---------------------------
            # Software pipeline over (chunk, pair) items: scores+exp
            # (stage A) runs DEPTH items ahead of AV/normalize (stage B).
            # Chunk ch's output projection is spread one s-tile per item
            # across chunk ch+1's items, so the PE always has independent
            # outproj work while the DVE drains the normalize chain.
            with (
                tc.tile_pool(name="avps", bufs=1, space="PSUM") as avps,
                tc.tile_pool(name="outps", bufs=2, space="PSUM") as outps,
                tc.tile_pool(name="scps", bufs=2, space="PSUM") as scps,
            ):
                def stage_b(ch, pt, ex):
                    # per head-pair: bank 0 of av = [AV0 rows 0-63 | AV1
                    # rows 64-127], bank 1 = [den0 | den1], built from
                    # col-tiled M=64 matmul pairs (col bases 0/64 run
                    # concurrently). One full-width reciprocal and one
                    # full-width mul then normalize both heads at once.
                    if pt == 0:
                        ho_tiles[ch] = p2ho.tile(
                            [128, NPT, SCH], BF16, name="ho_sb"
                        )
                    ho_sb = ho_tiles[ch]
                    av = avps.tile([128, 2, SCH], F32, name="av")
                    for kt in range(NDB):
                        st_ = (kt == 0)
                        sp_ = (kt == NDB - 1)
                        for hrow in range(2):
                            h0 = (2 * pt + hrow) * 64
                            lo = hrow * 64
                            nc.tensor.matmul(
                                av[lo:lo + 64, 0, :],
                                vp_sb[:, kt, h0:h0 + 64],
                                ex[:, hrow, kt, :],
                                start=st_, stop=sp_,
                                skip_group_check=True,
                            )
                        for hrow in range(2):
                            lo = hrow * 64
                            nc.tensor.matmul(
                                av[lo:lo + 64, 1, :],
                                ones_sb,
                                ex[:, hrow, kt, :],
                                start=st_, stop=sp_,
                                skip_group_check=True,
                            )
                    rc = p2rc.tile([128, SCH], F32, name="rc")
                    nc.vector.reciprocal_approx_fast(rc, av[:, 1, :])
                    nc.vector.tensor_mul(ho_sb[:, pt, :], av[:, 0, :], rc)

                def outproj_st(ch, st):
                    ho_sb = ho_tiles[ch]
                    osb = p2out.tile([128, C], BF16, name="osb")
                    row = ch * SCH + st * 128
                    for cc in range(2):
                        ops = outps.tile([128, 512], F32, name="ops")
                        for pt in range(NPT):
                            nc.tensor.matmul(
                                ops,
                                ho_sb[:, pt, st * 128:(st + 1) * 128],
                                wo_sb[:, pt, cc * 512:(cc + 1) * 512],
                                start=(pt == 0), stop=(pt == NPT - 1),
                            )
                        dst = osb[:, cc * 512:(cc + 1) * 512]
                        final = ch == NCH - 1 and st == NST - 1
                        # drains split DVE/ACT: Copy is in every ACT
                        # table set, so no table reload against Exp.
                        # The final tile also splits the DMA issue
                        # across the two HWDGE queues.
                        if cc == 1:
                            nc.scalar.copy(dst, ops)
                            eng = nc.scalar if final else nc.sync
                        else:
                            nc.vector.tensor_copy(dst, ops)
                            eng = nc.sync
                        eng.dma_start(
                            out[row:row + 128, cc * 512:(cc + 1) * 512],
                            dst,
                        )
                    if st == NST - 1:
                        ho_tiles.pop(ch)

                for i, (ch, pt) in enumerate(items):
                    if RAMP <= i + DEPTH < len(items):
                        ex_tiles[items[i + DEPTH]] = stage_a(
                            *items[i + DEPTH], scps, "scp"
                        )
                    if bi < len(items) and i < 3:
                        # leftover Q^T units as PE filler while the
                        # normalize chain fills (outps-pool scratch)
                        b_unit(bi, outps, "ops")
                        bi += 1
                    # outproj (independent PE work) is queued BEFORE
                    # stage_b: the PE runs its queue in order. The spread
                    # is shifted one item late so an item never runs an
                    # outproj whose ho normalize finished only one item
                    # ago: item (ch,0) runs the two-chunks-old final
                    # s-tile instead (needs p2ho bufs=3).
                    if pt == 0:
                        if ch >= 2:
                            outproj_st(ch - 2, NST - 1)
                    elif ch >= 1:
                        outproj_st(ch - 1, pt - 1)
                    stage_b(ch, pt, ex_tiles.pop((ch, pt)))
                outproj_st(NCH - 2, NST - 1)
                for st in range(NST):
                    outproj_st(NCH - 1, st)

    nc.compile()
    return nc


def get_compiled():
    global _compiled
    with _lock:
        if _compiled is None:
            _compiled = _build()
    return _compiled


def make_in_maps(x, Wq, Wk, Wv, E_w, E_b, F_w, F_b, Wo, bo):
    """Host-side sharding: core i -> (batch i//2, head-group i%2)."""
    import ml_dtypes

    f = np.float32
    bf = ml_dtypes.bfloat16
    x = np.asarray(x, f)
    efT = np.ascontiguousarray(
        np.concatenate([np.asarray(E_w, f).T, np.asarray(F_w, f).T], axis=1)
    ).astype(bf)                                        # [S, 2*DK]
    in_maps = []
    for core in range(NCORES):
        b, g = divmod(core, 2)
        hs = slice(g * HG, (g + 1) * HG)
        wq = np.asarray(Wq, f)[hs].reshape(HD, C)
        wk = np.asarray(Wk, f)[hs].reshape(HD, C)
        wv = np.asarray(Wv, f)[hs].reshape(HD, C)
        wo = np.asarray(Wo, f)[:, g * HD:(g + 1) * HD]      # [C, 512]
        in_maps.append({
            "xs": np.ascontiguousarray(x[b]).astype(bf),    # [S, C]
            "xt": np.ascontiguousarray(x[b].T).astype(bf),  # [C, S]
            "eft": efT,
            "wqt": np.ascontiguousarray(wq.T).astype(bf),   # [C, HD]
            "wkt": np.ascontiguousarray(wk.T).astype(bf),
            "wvt": np.ascontiguousarray(wv.T).astype(bf),
            "eb": np.asarray(E_b, f),
            "fb": np.asarray(F_b, f),
            "wot": np.ascontiguousarray(wo.T).astype(bf),   # [HD, C]
            "ones": np.ones((128, 64), bf),
        })
    return in_maps


def assemble(results, bo):
    out = np.empty((B, S, C), np.float32)
    for b in range(B):
        out[b] = (
            np.asarray(results[2 * b]["out"], np.float32)
            + np.asarray(results[2 * b + 1]["out"], np.float32)
        )
    out += np.asarray(bo, np.float32)[None, None, :]
    return out


def kernel(x, Wq, Wk, Wv, E_w, E_b, F_w, F_b, Wo, bo):
    from concourse.bass_utils import run_bass_kernel_spmd

    nc = get_compiled()
    in_maps = make_in_maps(x, Wq, Wk, Wv, E_w, E_b, F_w, F_b, Wo, bo)
    res = run_bass_kernel_spmd(nc, in_maps, core_ids=list(range(NCORES)))
    return assemble(res.results, bo)


# revision 43
# speedup vs baseline: 1.0123x; 1.0123x over previous
"""Linformer-style multi-head attention on 8 Trainium2 NeuronCores.

Problem (hardcoded): B=4, S=4096, C=1024, H=16, D=64, DK=256, fp32 in/out.

Sharding: core i handles (batch b = i//2, head-group g = i%2 of 8 heads).
Each core computes its 8 heads' attention and the partial output
projection out_part = head_out_g @ Wo[:, g_cols].T (emitted bf16); the
host sums the two head-group partials per batch and adds bo.

Key algebraic reordering: K and V are NEVER materialized. The Linformer
projections commute with the head projections:
    Kp = Wk @ (x^T E^T),  Vp^T = (x^T F^T)^T @ Wv^T
so we first compute xEF = x^T @ [E^T | F^T]  ([C, 2*DK], contraction
over the full sequence), then tiny [C]-contractions produce Kp [hd, dk]
and Vp^T [dk, hd]. This replaces the x@Wk / x@Wv GEMMs + sequence
reductions (328k PE cycles) with 131k + 16k.

Phase schedule (single stream).  The golden rule on TRN2 is that ANY
tensor-engine idle gap >~1us drops the HAM p-state to K=4/8 (half
throughput) for the next 10-27us, so every phase boundary is bridged
with independent PE work:
  warmup  16 throwaway matmuls on a memset tile so the p-state ramps
          while the first input DMAs land.
  A||B    A = xEF accumulation in two c-half passes (4 PSUM banks each,
          pass 2 reuses pass 1's banks after an interleaved drain);
          B = Q^T = Wq x^T per (chunk, head-pair).  Units interleave so
          HBM demand stays ~200-260 GB/s instead of phase-serial
          380/70; const loads are spread across units in first-use
          order.  ef stays SBUF-resident for pass 2; x streams once.
  C       Kp/Vp from xEF, between two B-units; bias-adds (DVE/ACT) are
          off the critical path.
  ramp    the software pipeline's DEPTH stage_a's are ACT-bound (two
          1.1us exps each), so each is paired with a leftover B-unit as
          PE filler; their scores use scratch PSUM in the qt pool.
  D       attention + output projection, software-pipelined over
          (chunk, head-pair) items: scoresT via row-group-packed K=64
          matmul pairs, exp on ACT, AV+softmax-den via col-tiled M=64
          matmul pairs, one full-width reciprocal+mul on DVE, outproj
          spread one s-tile per item one chunk late.  The last five
          B-units ride the first five items (outps-pool scratch) as
          filler while the normalize chain fills.  osb copies split
          DVE/ACT; out partials written bf16.
  PSUM: warm 1 + qt 3 | A 4 + qt 3 | kvps 4 + qt 3 | ramp qt 3 + scpw 4
  | D: avps 2 + outps 2 + scps 4.
"""

import threading

import numpy as np

B, S, C = 4, 4096, 1024
H, D, DK = 16, 64, 256
HG = 8               # heads per core
HD = HG * D          # 512
NCORES = 8
EF = 2 * DK          # stacked E^T|F^T columns: 512
SCH = 512            # sequence chunk (phase B / D)
NCH = S // SCH       # 8 chunks
NST = SCH // 128     # 4 s-tiles per chunk
NCT = C // 128       # 8 c-tiles
NPT = HD // 128      # 4 hd blocks (head pairs)
NDB = DK // 128      # 2 dk blocks
NSG = 8              # phase-A s-groups (512 rows each, 4-row interleave)
CH2 = C // 2         # phase-A column half
DEPTH = 4            # phase-D software-pipeline lookahead
RAMP = 5             # stage_a items issued during the rampup

_lock = threading.Lock()
_compiled = None


def _build():
    import concourse.bacc as bacc
    import concourse.bass as bass
    import concourse.tile as tile
    from concourse import mybir

    F32 = mybir.dt.float32
    BF16 = mybir.dt.bfloat16
    EXP = mybir.ActivationFunctionType.Exp

    nc = bacc.Bacc(None, target_bir_lowering=False)

    xs = nc.dram_tensor("xs", [S, C], BF16, kind="ExternalInput")
    xT = nc.dram_tensor("xt", [C, S], BF16, kind="ExternalInput")
    efT = nc.dram_tensor("eft", [S, EF], BF16, kind="ExternalInput")
    wqT = nc.dram_tensor("wqt", [C, HD], BF16, kind="ExternalInput")
    wkT = nc.dram_tensor("wkt", [C, HD], BF16, kind="ExternalInput")
    wvT = nc.dram_tensor("wvt", [C, HD], BF16, kind="ExternalInput")
    eb = nc.dram_tensor("eb", [DK], F32, kind="ExternalInput")
    fb = nc.dram_tensor("fb", [DK], F32, kind="ExternalInput")
    woT = nc.dram_tensor("wot", [HD, C], BF16, kind="ExternalInput")
    ones = nc.dram_tensor("ones", [128, 64], BF16, kind="ExternalInput")
    out = nc.dram_tensor("out", [S, C], BF16, kind="ExternalOutput")

    # phase-A x/ef tiles: partition p carries 4 consecutive s-rows (fat
    # contiguous DRAM reads per partition).  The s-permutation is
    # irrelevant: phase A only ever CONTRACTS over s, with x and ef
    # sharing the same permutation.
    xs_r = xs[:].rearrange("(g p four) c -> g p four c", p=128, four=4)
    ef_r = efT[:].rearrange("(g p four) k -> g p four k", p=128, four=4)
    xT_r = xT[:].rearrange("(ct p) s -> ct p s", p=128)     # [8,128,4096]
    wq_r = wqT[:].rearrange("(ct p) n -> ct p n", p=128)    # [8,128,512]
    wk_r = wkT[:].rearrange("(ct p) n -> ct p n", p=128)
    wv_r = wvT[:].rearrange("(ct p) n -> ct p n", p=128)
    wo_r = woT[:].rearrange("(pt p) c -> pt p c", p=128)    # [4,128,1024]

    with tile.TileContext(nc) as tc:
        with (
            tc.tile_pool(name="consts", bufs=1) as consts,
            tc.tile_pool(name="mids", bufs=1) as mids,
            tc.tile_pool(name="pbx", bufs=3) as pbx,
            tc.tile_pool(name="p2ex", bufs=5) as p2ex,
            tc.tile_pool(name="p2ho", bufs=3) as p2ho,
            tc.tile_pool(name="p2rc", bufs=1) as p2rc,
            tc.tile_pool(name="p2out", bufs=4) as p2out,
        ):
            warm_sb = consts.tile([128, 512], BF16)
            wq_sb = consts.tile([128, NCT, HD], BF16)
            wk_sb = consts.tile([128, NCT, HD], BF16)
            wv_sb = consts.tile([128, NCT, HD], BF16)
            eb_sb = consts.tile([128, DK], F32)
            fb_sb = consts.tile([128, NDB], F32)
            ones_sb = consts.tile([128, 64], BF16)

            ef_sb = mids.tile([128, NSG, 4, EF], BF16)  # resident E^T|F^T
            xef_sb = mids.tile([128, NCT, EF], BF16)    # [c, 2dk]
            kp_sb = mids.tile([128, NPT, DK], BF16)     # Kp [hd, dk]
            vp_sb = mids.tile([128, NDB, HD], BF16)     # Vp^T [dk, hd]
            qt_sb = mids.tile([128, NCH * NPT, SCH], BF16)  # Q^T, all chunks
            wo_sb = mids.tile([128, NPT, C], BF16)

            items = [(ch, pt) for ch in range(NCH) for pt in range(NPT)]
            ex_tiles = {}
            ho_tiles = {}

            def stage_a(ch, pt, pool, tname):
                # one 4-bank scp tile and ONE exp over all 2048 columns:
                # the second activation's ~450ns fixed cost was pushing
                # ACT to the pipeline pace.
                qt_c = qt_sb[:, ch * NPT + pt, :]
                ex = p2ex.tile([128, 2, NDB, SCH], BF16, name="ex")
                scp = pool.tile([128, 2, NDB, SCH], F32, name=tname, bufs=1)
                for hrow in range(2):
                    lo, hi = hrow * 64, (hrow + 1) * 64
                    for j in range(NDB):
                        nc.tensor.matmul(
                            scp[:, hrow, j, :],
                            kp_sb[lo:hi, pt, j * 128:(j + 1) * 128],
                            qt_c[lo:hi, :],
                            start=True, stop=True,
                        )
                nc.scalar.activation(ex[:], scp, EXP, scale=0.125)
                return ex

            # qtps footprint = 3 x 1 bank, coexisting with warmps(1) /
            # accA(4) / kvps(4) / rampps(4) under the 8-bank budget.
            with tc.tile_pool(name="qtps", bufs=3, space="PSUM") as qtps:
                # ---- warmup: PE busy while the first input DMAs land --
                with tc.tile_pool(name="warmps", bufs=1, space="PSUM") as wps:
                    warm_ps = wps.tile([128, 512], F32)
                    nc.vector.memset(warm_sb[:], 0.0)
                    # ~7us of PE busy: cold HBM can't feed the first A
                    # units before then, and idle-starved starts drop
                    # the HAM p-state for 10-20us.
                    for _ in range(32):
                        nc.tensor.matmul(
                            warm_ps, warm_sb[:, 0:128], warm_sb,
                            start=True, stop=True,
                        )

                # ---- DMA kickoff (only what phase start needs) -------
                nc.scalar.dma_start(ones_sb, ones[:])
                for f in range(4):
                    nc.sync.dma_start(ef_sb[:, 0, f, :], ef_r[0][:, f, :])
                a_tiles = {}

                def fetch_a(p, g):
                    if (p, g) in a_tiles or g >= NSG:
                        return a_tiles.get((p, g))
                    t = pax.tile([128, 4, CH2], BF16, name="xsg")
                    if p == 0 and g == 0:
                        for f in range(4):
                            nc.sync.dma_start(
                                t[:, f, :], xs_r[0][:, f, 0:CH2]
                            )
                    else:
                        nc.sync.dma_start(
                            t[:], xs_r[g][:, :, p * CH2:(p + 1) * CH2]
                        )
                    a_tiles[(p, g)] = t
                    return t

                # gpsimd: xT per chunk.  SW-DGE issue (~1us/desc)
                # staggers the chunks; pbx bufs=3 WAR-paces the rest
                # (gpsimd has nothing else to do).
                xt_tiles = []
                for ch in range(NCH):
                    xt_t = pbx.tile([128, NCT, SCH], BF16, name="xtc")
                    for ct in range(NCT):
                        nc.gpsimd.dma_start(
                            xt_t[:, ct, :],
                            xT_r[ct, :, ch * SCH:(ch + 1) * SCH],
                        )
                    xt_tiles.append(xt_t)

                # consts spread across units on scalar, first-use order:
                # (pass, group) -> list of dma closures
                feed = {}

                def _later(p, g, fn):
                    feed.setdefault((p, g), []).append(fn)

                for ct in range(NCT):
                    _later(0, ct // 4, lambda ct=ct: nc.scalar.dma_start(
                        wq_sb[:, ct, :], wq_r[ct]))
                for ct in range(NCT):
                    _later(0, 4 + ct // 2, lambda ct=ct: nc.scalar.dma_start(
                        wk_sb[:, ct, :], wk_r[ct]))
                for ct in range(NCT):
                    _later(1, ct // 2, lambda ct=ct: nc.scalar.dma_start(
                        wv_sb[:, ct, :], wv_r[ct]))
                eb_bc = bass.AP(tensor=eb[:].tensor, offset=0, ap=[[0, 128], [1, DK]])
                _later(1, 1, lambda: nc.scalar.dma_start(eb_sb[:], eb_bc))

                def _fb(db):
                    fb_col = fb[db * 128:(db + 1) * 128].rearrange(
                        "(p one) -> p one", one=1
                    )
                    nc.scalar.dma_start(fb_sb[:, db:db + 1], fb_col)

                _later(1, 2, lambda: _fb(0))
                _later(1, 2, lambda: _fb(1))
                for pt in range(NPT):
                    _later(1, 4 + pt, lambda pt=pt: nc.scalar.dma_start(
                        wo_sb[:, pt, :], wo_r[pt]))

                def a_unit(p, g, xef_ps):
                    x_t = fetch_a(p, g)
                    if p == 0 and 2 <= g + 2 < NSG:
                        nc.sync.dma_start(ef_sb[:, g + 2, :, :], ef_r[g + 2])
                    fetch_a(p, g + 2)
                    if p == 0 and g >= NSG - 2:
                        fetch_a(1, g - (NSG - 2))
                    for fn in feed.get((p, g), []):
                        fn()
                    last_g = g == NSG - 1
                    for f in range(4):
                        first = g == 0 and f == 0
                        last = last_g and f == 3
                        for ct in range(4):
                            nc.tensor.matmul(
                                xef_ps[:, ct, :],
                                x_t[:, f, ct * 128:(ct + 1) * 128],
                                ef_sb[:, g, f, :],
                                start=first, stop=last,
                            )
                            if last:
                                # interleaved drain: bank ct final here;
                                # copy while the PE continues ct+1..
                                dst = xef_sb[:, p * 4 + ct, :]
                                if ct % 2 == 0:
                                    nc.vector.tensor_copy(dst, xef_ps[:, ct, :])
                                else:
                                    nc.scalar.copy(dst, xef_ps[:, ct, :])
                    a_tiles.pop((p, g))

                def b_unit(idx, pool, tname, copy_eng=None):
                    ch, pt = idx // NPT, idx % NPT
                    xt_t = xt_tiles[ch]
                    qps = pool.tile([128, SCH], F32, name=tname)
                    for ct in range(NCT):
                        nc.tensor.matmul(
                            qps,
                            wq_sb[:, ct, pt * 128:(pt + 1) * 128],
                            xt_t[:, ct, :],
                            start=(ct == 0), stop=(ct == NCT - 1),
                        )
                    dst = qt_sb[:, ch * NPT + pt, :]
                    if copy_eng is nc.scalar:
                        nc.scalar.copy(dst, qps)
                    else:
                        nc.vector.tensor_copy(dst, qps)

                # B-unit budget: 21 in A||B, 2 around C, 5 in the D
                # rampup, 4 spread into D's first items.  Pass-1 units
                # carry 3 B each (1A:3B keeps pass-1 HBM demand at
                # ~240 GB/s — pass 1 moves 2/3 of phase A's bytes);
                # pass 2 runs nearly A-only at ~150 GB/s.
                npair = {0: {1: 1, 2: 3, 3: 3, 4: 3, 5: 3, 6: 3, 7: 3},
                         1: {0: 1, 1: 1}}
                bi = 0
                with (
                    tc.tile_pool(name="pax", bufs=3) as pax,
                    tc.tile_pool(name="accA", bufs=1, space="PSUM") as accA,
                ):
                    fetch_a(0, 0)
                    fetch_a(0, 1)
                    nc.sync.dma_start(ef_sb[:, 1, :, :], ef_r[1])
                    for p in range(2):
                        xef_ps = accA.tile([128, 4, EF], F32, name="xefps")
                        for g in range(NSG):
                            a_unit(p, g, xef_ps)
                            for _ in range(npair[p].get(g, 0)):
                                b_unit(bi, qtps, "qps")
                                bi += 1

                # ---- C: Kp/Vp, bracketed by B-units ------------------
                with tc.tile_pool(name="kvps", bufs=1, space="PSUM") as kvps:
                    kp_ps = kvps.tile([128, NPT, DK], F32)      # 2 banks
                    vp_ps = kvps.tile([128, NDB, HD], F32)      # 2 banks
                    b_unit(bi, qtps, "qps")
                    bi += 1
                    for pt in range(NPT):
                        for ct in range(NCT):
                            nc.tensor.matmul(
                                kp_ps[:, pt, :],
                                wk_sb[:, ct, pt * 128:(pt + 1) * 128],
                                xef_sb[:, ct, 0:DK],
                                start=(ct == 0 and pt % 2 == 0),
                                stop=(ct == NCT - 1 and pt % 2 == 1),
                            )
                    for pt in range(NPT):
                        nc.vector.tensor_add(
                            kp_sb[:, pt, :], kp_ps[:, pt, :], eb_sb
                        )
                    b_unit(bi, qtps, "qps")
                    bi += 1
                    for db in range(NDB):
                        for ct in range(NCT):
                            nc.tensor.matmul(
                                vp_ps[:, db, :],
                                xef_sb[:, ct, DK + db * 128:DK + (db + 1) * 128],
                                wv_sb[:, ct, :],
                                start=(ct == 0), stop=(ct == NCT - 1),
                            )
                    for db in range(NDB):
                        # fb varies along the partition (dk) axis: ACT's
                        # per-partition bias-add fits, keeping the DVE
                        # free for the qt copies.
                        nc.scalar.add(
                            vp_sb[:, db, :], vp_ps[:, db, :],
                            fb_sb[:, db:db + 1],
                        )

                # ---- D rampup: stage_a is ACT-bound (~2us exp), so
                # pair each with a B-unit as PE filler; the B-unit
                # covers the previous item's exp (scp bufs=1).  Scores
                # use a dedicated 4-bank pool that closes before D's
                # scps opens; RAMP=5 so the main loop's first scps use
                # is one full item after the last ramp exp.
                with tc.tile_pool(name="rampps", bufs=1, space="PSUM") as rps:
                    for r in range(RAMP):
                        b_unit(bi, qtps, "qps")
                        bi += 1
                        ex_tiles[items[r]] = stage_a(*items[r], rps, "scpw")

            # ---------------- phase D main loop ---------------------------
            # Software pipeline over (chunk, pair) items: scores+exp
            # (stage A) runs DEPTH items ahead of AV/normalize (stage B).
            # Chunk ch's output projection is spread one s-tile per item
            # across chunk ch+1's items, so the PE always has independent
            # outproj work while the DVE drains the normalize chain.
            with (
                tc.tile_pool(name="avps", bufs=1, space="PSUM") as avps,
                tc.tile_pool(name="outps", bufs=2, space="PSUM") as outps,
                tc.tile_pool(name="scps", bufs=2, space="PSUM") as scps,
            ):
                def stage_b(ch, pt, ex):
                    # per head-pair: bank 0 of av = [AV0 rows 0-63 | AV1
                    # rows 64-127], bank 1 = [den0 | den1], built from
                    # col-tiled M=64 matmul pairs (col bases 0/64 run
                    # concurrently). One full-width reciprocal and one
                    # full-width mul then normalize both heads at once.
                    if pt == 0:
                        ho_tiles[ch] = p2ho.tile(
                            [128, NPT, SCH], BF16, name="ho_sb"
                        )
                    ho_sb = ho_tiles[ch]
                    av = avps.tile([128, 2, SCH], F32, name="av")
                    for kt in range(NDB):
                        st_ = (kt == 0)
                        sp_ = (kt == NDB - 1)
                        for hrow in range(2):
                            h0 = (2 * pt + hrow) * 64
                            lo = hrow * 64
                            nc.tensor.matmul(
                                av[lo:lo + 64, 0, :],
                                vp_sb[:, kt, h0:h0 + 64],
                                ex[:, hrow, kt, :],
                                start=st_, stop=sp_,
                                skip_group_check=True,
                            )
                        for hrow in range(2):
                            lo = hrow * 64
                            nc.tensor.matmul(
                                av[lo:lo + 64, 1, :],
                                ones_sb,
                                ex[:, hrow, kt, :],
                                start=st_, stop=sp_,
                                skip_group_check=True,
                            )
                    rc = p2rc.tile([128, SCH], F32, name="rc")
                    nc.vector.reciprocal_approx_fast(rc, av[:, 1, :])
                    nc.vector.tensor_mul(ho_sb[:, pt, :], av[:, 0, :], rc)

                def outproj_st(ch, st):
                    ho_sb = ho_tiles[ch]
                    osb = p2out.tile([128, C], BF16, name="osb")
                    row = ch * SCH + st * 128
                    for cc in range(2):
                        ops = outps.tile([128, 512], F32, name="ops")
                        for pt in range(NPT):
                            nc.tensor.matmul(
                                ops,
                                ho_sb[:, pt, st * 128:(st + 1) * 128],
                                wo_sb[:, pt, cc * 512:(cc + 1) * 512],
                                start=(pt == 0), stop=(pt == NPT - 1),
                            )
                        dst = osb[:, cc * 512:(cc + 1) * 512]
                        final = ch == NCH - 1 and st == NST - 1
                        # drains split DVE/ACT: Copy is in every ACT
                        # table set, so no table reload against Exp.
                        # The final tile also splits the DMA issue
                        # across the two HWDGE queues.
                        if cc == 1:
                            nc.scalar.copy(dst, ops)
                            eng = nc.scalar if final else nc.sync
                        else:
                            nc.vector.tensor_copy(dst, ops)
                            eng = nc.sync
                        eng.dma_start(
                            out[row:row + 128, cc * 512:(cc + 1) * 512],
                            dst,
                        )
                    if st == NST - 1:
                        ho_tiles.pop(ch)

                for i, (ch, pt) in enumerate(items):
                    if RAMP <= i + DEPTH < len(items):
                        ex_tiles[items[i + DEPTH]] = stage_a(
                            *items[i + DEPTH], scps, "scp"
                        )
                    if bi < len(items) and i < 4:
                        # leftover Q^T units as PE filler while the
                        # normalize chain fills (outps-pool scratch).
                        # Their qt copies ride ACT: early items have no
                        # outproj PE work, so the DVE's recip+mul chain
                        # is the pacer and an extra DVE copy starves
                        # the PE (~0.45us/item measured).
                        b_unit(bi, outps, "ops", copy_eng=nc.scalar)
                        bi += 1
                    # outproj (independent PE work) is queued BEFORE
                    # stage_b: the PE runs its queue in order. The spread
                    # is shifted one item late so an item never runs an
                    # outproj whose ho normalize finished only one item
                    # ago: item (ch,0) runs the two-chunks-old final
                    # s-tile instead (needs p2ho bufs=3).
                    if pt == 0:
                        if ch >= 2:
                            outproj_st(ch - 2, NST - 1)
                    elif ch >= 1:
                        outproj_st(ch - 1, pt - 1)
                    stage_b(ch, pt, ex_tiles.pop((ch, pt)))
                outproj_st(NCH - 2, NST - 1)
                for st in range(NST):
                    outproj_st(NCH - 1, st)

    nc.compile()
    return nc


def get_compiled():
    global _compiled
    with _lock:
        if _compiled is None:
            _compiled = _build()
    return _compiled


def make_in_maps(x, Wq, Wk, Wv, E_w, E_b, F_w, F_b, Wo, bo):
    """Host-side sharding: core i -> (batch i//2, head-group i%2)."""
    import ml_dtypes

    f = np.float32
    bf = ml_dtypes.bfloat16
    x = np.asarray(x, f)
    efT = np.ascontiguousarray(
        np.concatenate([np.asarray(E_w, f).T, np.asarray(F_w, f).T], axis=1)
    ).astype(bf)                                        # [S, 2*DK]
    in_maps = []
    for core in range(NCORES):
        b, g = divmod(core, 2)
        hs = slice(g * HG, (g + 1) * HG)
        wq = np.asarray(Wq, f)[hs].reshape(HD, C)
        wk = np.asarray(Wk, f)[hs].reshape(HD, C)
        wv = np.asarray(Wv, f)[hs].reshape(HD, C)
        wo = np.asarray(Wo, f)[:, g * HD:(g + 1) * HD]      # [C, 512]
        in_maps.append({
            "xs": np.ascontiguousarray(x[b]).astype(bf),    # [S, C]
            "xt": np.ascontiguousarray(x[b].T).astype(bf),  # [C, S]
            "eft": efT,
            "wqt": np.ascontiguousarray(wq.T).astype(bf),   # [C, HD]
            "wkt": np.ascontiguousarray(wk.T).astype(bf),
            "wvt": np.ascontiguousarray(wv.T).astype(bf),
            "eb": np.asarray(E_b, f),
            "fb": np.asarray(F_b, f),
            "wot": np.ascontiguousarray(wo.T).astype(bf),   # [HD, C]
            "ones": np.ones((128, 64), bf),
        })
    return in_maps


def assemble(results, bo):
    out = np.empty((B, S, C), np.float32)
    for b in range(B):
        out[b] = (
            np.asarray(results[2 * b]["out"], np.float32)
            + np.asarray(results[2 * b + 1]["out"], np.float32)
        )
    out += np.asarray(bo, np.float32)[None, None, :]
    return out


def kernel(x, Wq, Wk, Wv, E_w, E_b, F_w, F_b, Wo, bo):
    from concourse.bass_utils import run_bass_kernel_spmd

    nc = get_compiled()
    in_maps = make_in_maps(x, Wq, Wk, Wv, E_w, E_b, F_w, F_b, Wo, bo)
    res = run_bass_kernel_spmd(nc, in_maps, core_ids=list(range(NCORES)))
    return assemble(res.results, bo)
